# revision 56
# baseline (speedup 1.0000x reference)
"""FBP reconstructor on 8 TRN2 NeuronCores (Bass/Tile).

Pipeline (per core, angle-sharded, class-pure fast-axis):
  1. column sums of x with H-boundary masks  (PE matmul, colmask const)
  2. 9-tap 1D conv -> proj_sim^T [det, (b,a)] (PE matmul)
  3. Ram-Lak filter as two DFT matmuls with constant cos/sin matrices;
     the inverse-DFT is evaluated twice, once with column-reversed
     basis matrices, giving detector-REVERSED filtered rows
  4. per-octet table extraction via selector matmuls: each
     16-partition group = one angle pair x {4 batches x 2 mirror
     halves} x {normal, reversed table}.  Quartet symmetry: the same
     (floor ix, frac) stream evaluates pixel (x,y) [T, s=0],
     (255-x,y) [partner T, s=1], (255-x,255-y) [reversed T] and
     (x,255-y) [partner reversed T] -- one gather serves 4 pixels.
  5. interleaved fp16 segment tables TAB[k] = (T[k], E0..E2[k]) with
     E0 = D[k], Er = D[k+r]-D[k+r-1], D[k] = T[k+1]-T[k]
  6. cores 0-3 take x-fast pairs (45..89, row-major pixels), cores
     4-7 y-fast (0..44, column-major): fast-axis step <= 0.71 so
     3 relu taps suffice (2-word gather entries; the six ~45-degree
     pairs accept a <=0.11*E3 tap-3 truncation).  oct2 = the 4 most
     extreme pairs (step <= 0.27) at 8-px segments.
  7. ap_gather per segment; interp val = T[a] + sum_r relu(pf-r)*Er:
     relu chain on ScalarE (relu(pf-r) = relu(relu(pf-r+1)-1)),
     batched per-pixel expansion of Er columns on ScalarE, contiguous
     fp16 multiplies on DVE, T-term folded with a broadcast add
  8. PE selector-matmul accumulation into PSUM [16, 2048]; PSUM
     eviction is software-pipelined one iteration behind (fp16)

Host: builds constants, per-segment anchors and fp16 pfrac tiles,
merges the 8 partial outputs (transpose for y-fast cores, axis flips
for the symmetry quartet) and scales.
"""

import sys

if "/opt/trn_rl_repo" not in sys.path:
    sys.path.insert(0, "/opt/trn_rl_repo")

import numpy as np

IMG = 256
DET = 256
NA = 180
NB = 4
NFREQ = DET // 2 + 1  # 129
SEG = 4  # oct1 pixels per gather index
NTAP1 = 3  # oct1 relu taps (fast-axis step <= 0.707; tiny tap-3
           # truncation accepted for the six ~45-degree pairs)
DW = 4  # oct1 gather window: (T, E0, E1, E2)
SEG2 = 8  # oct2 pixels per gather index (extreme-angle pairs, step <= 0.27)
NTAP2 = 3  # oct2 relu taps (span 7*0.27 < 2)
DW2 = 4  # oct2 gather window: (T, E0, E1, E2)
CHUNK = 2048
NSEG = CHUNK // SEG  # 512 segments per chunk
NSEG2 = CHUNK // SEG2  # 256 oct2 segments per chunk
NCHP = 16  # chunks processed on device (y-flip covers 16..31)
NCC = 8  # oct2 sweep iterations
NPAIR = NA // 2  # 90 flip-sym base pairs
NCORES = 8
BA = NB * NA  # 720
MT = 120  # ba M-tile size (6 tiles)

_cache = {}


def _host_constants():
    """Input-independent constants."""
    colmask = np.ones((IMG, 3), dtype=np.float32)
    colmask[IMG - 1, 0] = 0.0
    colmask[0, 2] = 0.0

    n = np.arange(DET)[:, None]
    k = np.arange(NFREQ)[None, :]
    phase = 2.0 * np.pi * n * k / DET
    dftAc = np.cos(phase).astype(np.float32)  # [256,129]
    dftAs = np.sin(phase).astype(np.float32)
    u = np.full(NFREQ, 2.0 / DET, dtype=np.float64)
    u[0] = 1.0 / DET
    u[NFREQ - 1] = 1.0 / DET
    m = np.arange(DET)[None, :]
    kk = np.arange(NFREQ)[:, None]
    phB = 2.0 * np.pi * kk * m / DET
    dftBc = (u[:, None] * np.cos(phB)).astype(np.float32)  # [129,256]
    dftBs = (u[:, None] * np.sin(phB)).astype(np.float32)
    dftBcr = np.ascontiguousarray(dftBc[:, ::-1])  # reversed detector
    dftBsr = np.ascontiguousarray(dftBs[:, ::-1])

    # accumulation selectors [3][128, 16]: row 16g+c -> col c (c<16),
    # filtered by group parity for the two sweep-2 variants.
    sela = np.zeros((3, 128, 16), dtype=np.float32)
    for g in range(8):
        for c in range(16):
            sela[0, 16 * g + c, c] = 1.0
            if g % 2 == 0:
                sela[1, 16 * g + c, c] = 1.0
            else:
                sela[2, 16 * g + c, c] = 1.0

    # per-pair interp positions ix [NPAIR, IMG*IMG] (f32, matches ref)
    lin = np.linspace(-1.0, 1.0, IMG).astype(np.float32)
    yy, xx = np.meshgrid(lin, lin, indexing="ij")
    ang = np.linspace(0.0, np.pi, NA).astype(np.float32)
    ix_all = np.empty((NPAIR, IMG, IMG), dtype=np.float32)
    for i in range(NPAIR):
        c = np.float32(np.cos(ang[i]))
        s = np.float32(np.sin(ang[i]))
        t = (c * xx + s * yy).astype(np.float32)
        ix_all[i] = np.clip(
            (t + np.float32(1.0)) * np.float32(0.5) * np.float32(DET - 1),
            0.0, DET - 1).astype(np.float32)
    return (colmask, dftAc, dftAs, dftBc, dftBs, dftBcr, dftBsr,
            sela, ix_all)


def _wrap16(flat_idx):
    """[nseg] seg idx -> [16, nseg//16] wrapped (i at partition i%16, col i//16)."""
    return flat_idx.reshape(-1, 16).T


def _anchors_pfrac(ix, segw=SEG):
    """ix [CHUNK] f32 -> (anchors [CHUNK//segw] int16, pfrac [CHUNK] f16)."""
    seg = ix.reshape(-1, segw)
    anch = np.floor(seg.min(axis=1)).astype(np.int32)
    pf = (seg.astype(np.float64) - anch[:, None]).astype(np.float16)
    return anch.astype(np.int16), pf.reshape(-1)


def _sel_pair(p1, p2):
    """Selector matrices routing filtered rows to partitions.

    Returns (sel1n, sel1r, sel2n, sel2r): n = normal (partitions
    16g+4s+b), r = reversed-table copies (partitions 16g+8+4s+b).
    """
    def fill(sel, g, pi, hi):
        for s in range(2):
            a = pi if s == 0 else NA - 1 - pi
            for b in range(NB):
                sel[b * NA + a, 16 * g + 8 * hi + 4 * s + b] = 1.0

    sel1n = np.zeros((BA, 128), dtype=np.float16)
    sel1r = np.zeros((BA, 128), dtype=np.float16)
    for g, pi in enumerate(p1):
        if pi < 0:
            continue
        fill(sel1n, g, pi, 0)
        fill(sel1r, g, pi, 1)
    sel2n = np.zeros((BA, 128), dtype=np.float16)
    sel2r = np.zeros((BA, 128), dtype=np.float16)
    for p, pi in enumerate(p2):
        if pi < 0:
            continue
        for h in range(2):
            fill(sel2n, 2 * p + h, pi, 0)
            fill(sel2r, 2 * p + h, pi, 1)
    return sel1n, sel1r, sel2n, sel2r


def _core_pairs(r):
    """Class-pure pair assignment: cores 0-3 x-fast (pairs 45..89,
    row-major traversal), cores 4-7 y-fast (pairs 0..44, column-major).
    oct2 = the 4 most extreme (smallest fast-step) pairs of the core."""
    if r < 4:
        pairs = list(range(45 + r, NPAIR, 4))
        p2 = pairs[-4:]
        p1 = pairs[:-4]
    else:
        pairs = list(range(r - 4, 45, 4))
        p2 = pairs[:4]
        p1 = pairs[4:]
    while len(p1) < 8:
        p1.append(-1)
    while len(p2) < 4:
        p2.append(-1)
    return p1, p2


def _per_core_inputs(ix_all):
    """Per-core SEL matrices, wrapped anchor tiles and fp16 pfrac tiles."""
    out = []
    for r in range(NCORES):
        p1, p2 = _core_pairs(r)

        sels = _sel_pair(p1, p2)

        def stream(pi):
            img = ix_all[pi]
            if r >= 4:
                img = img.T  # y-fast: column-major traversal
            return np.ascontiguousarray(img).reshape(-1)

        # precompute anchors/pfrac per (pair, chunk) for chunks 0..15
        apf1 = {}
        for pi in p1:
            if pi < 0:
                continue
            st = stream(pi)
            for ci in range(NCHP):
                apf1[(pi, ci)] = _anchors_pfrac(
                    st[ci * CHUNK:(ci + 1) * CHUNK], SEG)
        apf2 = {}
        for pi in p2:
            if pi < 0:
                continue
            st = stream(pi)
            for ci in range(NCHP):
                apf2[(pi, ci)] = _anchors_pfrac(
                    st[ci * CHUNK:(ci + 1) * CHUNK], SEG2)

        idx1 = np.zeros((NCHP, 128, NSEG // 16), dtype=np.int16)
        pf1 = np.zeros((NCHP, 128, CHUNK), dtype=np.float16)
        for ci in range(NCHP):
            for g, pi in enumerate(p1):
                if pi < 0:
                    continue
                anch, pf = apf1[(pi, ci)]
                idx1[ci, 16 * g:16 * g + 16, :] = _wrap16(anch)
                pf1[ci, 16 * g:16 * g + 16, :] = pf[None, :]
        idx2 = np.zeros((NCC, 128, NSEG2 // 16), dtype=np.int16)
        pf2 = np.zeros((NCC, 128, CHUNK), dtype=np.float16)
        for cc in range(NCC):
            for p, pi in enumerate(p2):
                if pi < 0:
                    continue
                for h in range(2):
                    ci = 2 * cc + h
                    g = 2 * p + h
                    anch, pf = apf2[(pi, ci)]
                    idx2[cc, 16 * g:16 * g + 16, :] = _wrap16(anch)
                    pf2[cc, 16 * g:16 * g + 16, :] = pf[None, :]
        out.append(sels + (idx1, idx2,
                           pf1.reshape(NCHP, 128, NSEG, SEG),
                           pf2.reshape(NCC, 128, NSEG2, SEG2)))
    return out


def _build_nc():
    import concourse.mybir as mybir
    import concourse.tile as tile
    from concourse import bacc
    from concourse.bass import AP, broadcast_tensor_aps

    f32 = mybir.dt.float32
    f16 = mybir.dt.float16
    i16 = mybir.dt.int16
    Act = mybir.ActivationFunctionType

    nc = bacc.Bacc(None, target_bir_lowering=False, debug=False)

    x_d = nc.dram_tensor("x3", [NB, IMG, IMG], f32, kind="ExternalInput")
    w9_d = nc.dram_tensor("w9", [3, 3, NA], f32, kind="ExternalInput")
    rcol_d = nc.dram_tensor("rcol", [128, 1], f32, kind="ExternalInput")
    rnyq_d = nc.dram_tensor("rnyq", [1, 1], f32, kind="ExternalInput")
    cm_d = nc.dram_tensor("colmask", [IMG, 3], f32, kind="ExternalInput")
    dAc_d = nc.dram_tensor("dftAc", [DET, NFREQ], f32, kind="ExternalInput")
    dAs_d = nc.dram_tensor("dftAs", [DET, NFREQ], f32, kind="ExternalInput")
    dBc_d = nc.dram_tensor("dftBc", [NFREQ, DET], f32, kind="ExternalInput")
    dBs_d = nc.dram_tensor("dftBs", [NFREQ, DET], f32, kind="ExternalInput")
    dBcr_d = nc.dram_tensor("dftBcr", [NFREQ, DET], f32, kind="ExternalInput")
    dBsr_d = nc.dram_tensor("dftBsr", [NFREQ, DET], f32, kind="ExternalInput")
    sela_d = nc.dram_tensor("sela", [128, 3, 16], f32, kind="ExternalInput")
    s1n_d = nc.dram_tensor("sel1n", [BA, 128], f16, kind="ExternalInput")
    s1r_d = nc.dram_tensor("sel1r", [BA, 128], f16, kind="ExternalInput")
    s2n_d = nc.dram_tensor("sel2n", [BA, 128], f16, kind="ExternalInput")
    s2r_d = nc.dram_tensor("sel2r", [BA, 128], f16, kind="ExternalInput")
    idx1_d = nc.dram_tensor("idx1", [NCHP, 128, NSEG // 16], i16,
                            kind="ExternalInput")
    idx2_d = nc.dram_tensor("idx2", [NCC, 128, NSEG2 // 16], i16,
                            kind="ExternalInput")
    pf1_d = nc.dram_tensor("pf1", [NCHP, 128, NSEG, SEG], f16,
                           kind="ExternalInput")
    pf2_d = nc.dram_tensor("pf2", [NCC, 128, NSEG2, SEG2], f16,
                           kind="ExternalInput")
    out_d = nc.dram_tensor("partial", [NCHP, 16, CHUNK], f16,
                           kind="ExternalOutput")

    with tile.TileContext(nc) as tc:
        with (
            tc.tile_pool(name="persist", bufs=1) as pp,
            tc.tile_pool(name="tab", bufs=1) as tabp,
        ):
            # ---- persistent small tiles
            sela_t = pp.tile([128, 3, 16], f32)
            rcol_t = pp.tile([128, 1], f32)
            rnyq_t = pp.tile([1, 1], f32)
            nc.sync.dma_start(out=sela_t[:], in_=sela_d[:, :, :])
            selab_t = pp.tile([128, 3, 16], f16)
            nc.scalar.copy(out=selab_t[:], in_=sela_t[:])
            nc.sync.dma_start(out=rcol_t[:], in_=rcol_d[:, :])
            nc.sync.dma_start(out=rnyq_t[:], in_=rnyq_d[:, :])

            # interleaved segment tables: oct1 (T, E0..E3, pad), oct2 (T, E0..E2)
            TAB1_t = tabp.tile([128, DET, DW], f16)
            TAB2_t = tabp.tile([128, DET, DW2], f16)
            # per-partition bias columns for relu taps: bias_t[:, r] = -r
            bias_t = pp.tile([128, NTAP2], f32)
            for r in range(NTAP2):
                nc.vector.memset(bias_t[:, r:r + 1], float(-r))

            with tc.tile_pool(name="setup", bufs=1) as sp:
                w9_t = sp.tile([3, 3, NA], f32)
                nc.sync.dma_start(out=w9_t[:], in_=w9_d[:, :, :])
                cm_t = sp.tile([128, 2, 3], f32)
                nc.sync.dma_start(out=cm_t[:, 0], in_=cm_d[0:128, :])
                nc.sync.dma_start(out=cm_t[:, 1], in_=cm_d[128:256, :])
                dAc_t = sp.tile([128, 2, NFREQ], f32)
                dAs_t = sp.tile([128, 2, NFREQ], f32)
                dB_t = sp.tile([128, 4, DET], f32)
                dBn_t = sp.tile([1, 4, DET], f32)
                dAc16 = sp.tile([128, 2, NFREQ], f16)
                dAs16 = sp.tile([128, 2, NFREQ], f16)
                dB16 = sp.tile([128, 4, DET], f16)
                dBn16 = sp.tile([1, 4, DET], f16)
                s1n_t = sp.tile([MT, 6, 128], f16)
                s1r_t = sp.tile([MT, 6, 128], f16)
                s2n_t = sp.tile([MT, 6, 128], f16)
                s2r_t = sp.tile([MT, 6, 128], f16)
                nc.sync.dma_start(out=dAc_t[:, 0], in_=dAc_d[0:128, :])
                nc.sync.dma_start(out=dAc_t[:, 1], in_=dAc_d[128:256, :])
                nc.sync.dma_start(out=dAs_t[:, 0], in_=dAs_d[0:128, :])
                nc.sync.dma_start(out=dAs_t[:, 1], in_=dAs_d[128:256, :])
                for sl, d_ in ((0, dBc_d), (1, dBs_d), (2, dBcr_d),
                               (3, dBsr_d)):
                    nc.sync.dma_start(out=dB_t[:, sl], in_=d_[0:128, :])
                    nc.sync.dma_start(out=dBn_t[:, sl], in_=d_[128:129, :])
                for t in range(6):
                    nc.sync.dma_start(out=s1n_t[:, t],
                                      in_=s1n_d[MT * t:MT * (t + 1), :])
                    nc.sync.dma_start(out=s1r_t[:, t],
                                      in_=s1r_d[MT * t:MT * (t + 1), :])
                    nc.sync.dma_start(out=s2n_t[:, t],
                                      in_=s2n_d[MT * t:MT * (t + 1), :])
                    nc.sync.dma_start(out=s2r_t[:, t],
                                      in_=s2r_d[MT * t:MT * (t + 1), :])

                pT_sb = sp.tile([128, 2, BA], f16)  # proj_sim^T [det, (b,a)]
                nc.scalar.copy(out=dAc16[:], in_=dAc_t[:])
                nc.scalar.copy(out=dAs16[:], in_=dAs_t[:])
                nc.scalar.copy(out=dB16[:], in_=dB_t[:])
                nc.scalar.copy(out=dBn16[:], in_=dBn_t[:])

                with (
                    tc.tile_pool(name="xload", bufs=2) as xp,
                    tc.tile_pool(name="ps_a", bufs=2, space="PSUM") as psa,
                ):
                    for b in range(NB):
                        xt = xp.tile([128, 2, IMG], f32)
                        nc.sync.dma_start(out=xt[:, 0], in_=x_d[b, 0:128, :])
                        nc.sync.dma_start(out=xt[:, 1], in_=x_d[b, 128:256, :])
                        tk_ps = psa.tile([3, IMG], f32)
                        nc.tensor.matmul(tk_ps[:], cm_t[:, 0], xt[:, 0],
                                         start=True, stop=False)
                        nc.tensor.matmul(tk_ps[:], cm_t[:, 1], xt[:, 1],
                                         start=False, stop=True)
                        # zero-padded column sums: tk3[_, 1+v] = Tk[v]
                        tk3_t = xp.tile([3, IMG + 2], f32)
                        nc.vector.memset(tk3_t[:], 0.0)
                        nc.scalar.copy(out=tk3_t[:, 1:IMG + 1], in_=tk_ps[:])
                        for jt in range(2):
                            pc_ps = psa.tile([128, NA], f32)
                            for kj in range(3):
                                nc.tensor.matmul(
                                    pc_ps[:],
                                    tk3_t[:, jt * 128 + kj:jt * 128 + kj + 128],
                                    w9_t[:, kj],
                                    start=(kj == 0), stop=(kj == 2))
                            nc.scalar.copy(
                                out=pT_sb[:, jt, b * NA:(b + 1) * NA],
                                in_=pc_ps[:])

                # ---- stage 1: spectra [freq, ba], ramlak-scaled
                spc_sb = sp.tile([128, BA], f16)
                sps_sb = sp.tile([128, BA], f16)
                spcn_sb = sp.tile([1, BA], f16)
                spsn_sb = sp.tile([1, BA], f16)
                with tc.tile_pool(name="ps_b", bufs=2, space="PSUM") as psb:
                    for var, dA_t, dst, dstn in (
                        (0, dAc16, spc_sb, spcn_sb),
                        (1, dAs16, sps_sb, spsn_sb),
                    ):
                        sp_ps = psb.tile([128, 1024], f32)
                        spn_ps = psb.tile([1, 1024], f32)
                        for ns in (slice(0, 512), slice(512, BA)):
                            for kt in range(2):
                                nc.tensor.matmul(
                                    sp_ps[:, ns], dA_t[:, kt, 0:128],
                                    pT_sb[:, kt, ns],
                                    start=(kt == 0), stop=(kt == 1))
                                nc.tensor.matmul(
                                    spn_ps[:, ns], dA_t[:, kt, 128:129],
                                    pT_sb[:, kt, ns],
                                    start=(kt == 0), stop=(kt == 1))
                        nc.vector.tensor_scalar_mul(dst[:], sp_ps[:, 0:BA],
                                                    rcol_t[:, 0:1])
                        nc.vector.tensor_scalar_mul(dstn[:], spn_ps[:, 0:BA],
                                                    rnyq_t[:, 0:1])

                # ---- stage 2: filtered [ba-tiles, det], normal + reversed
                filt_sb = sp.tile([MT, 2, 6, DET], f16)
                with tc.tile_pool(name="ps_c", bufs=2, space="PSUM") as psc:
                    for v in range(2):  # 0 normal, 1 reversed
                        for mt in range(6):
                            ms = slice(mt * MT, (mt + 1) * MT)
                            f_ps = psc.tile([MT, DET], f32)
                            nc.tensor.matmul(f_ps[:], spc_sb[:, ms],
                                             dB16[:, 2 * v],
                                             start=True, stop=False)
                            nc.tensor.matmul(f_ps[:], spcn_sb[:, ms],
                                             dBn16[:, 2 * v],
                                             start=False, stop=False)
                            nc.tensor.matmul(f_ps[:], sps_sb[:, ms],
                                             dB16[:, 2 * v + 1],
                                             start=False, stop=False)
                            nc.tensor.matmul(f_ps[:], spsn_sb[:, ms],
                                             dBn16[:, 2 * v + 1],
                                             start=False, stop=True)
                            nc.scalar.copy(out=filt_sb[:, v, mt], in_=f_ps[:])

                # ---- per-octet tables T -> TAB (T, E0..E3)
                with tc.tile_pool(name="ps_d", bufs=2, space="PSUM") as psd:
                    for TAB_o, ntap_o, selN_t, selR_t in (
                        (TAB2_t, NTAP2, s2n_t, s2r_t),
                        (TAB1_t, NTAP1, s1n_t, s1r_t),
                    ):
                        t_ps = psd.tile([128, DET], f32)
                        for kt in range(6):
                            nc.tensor.matmul(t_ps[:], selN_t[:, kt, :],
                                             filt_sb[:, 0, kt, :],
                                             start=(kt == 0), stop=False)
                        for kt in range(6):
                            nc.tensor.matmul(t_ps[:], selR_t[:, kt, :],
                                             filt_sb[:, 1, kt, :],
                                             start=False, stop=(kt == 5))
                        T_sb = sp.tile([128, DET], f32, tag="tsb")
                        nc.scalar.copy(out=T_sb[:], in_=t_ps[:])
                        # padded first differences Dp[k] = T[k+1]-T[k],
                        # zero for k >= DET-1 (and the window pad slots)
                        Dp_sb = sp.tile([128, DET + 8], f32, tag="dpsb")
                        nc.vector.memset(Dp_sb[:], 0.0)
                        nc.vector.tensor_sub(Dp_sb[:, 0:DET - 1],
                                             T_sb[:, 1:DET],
                                             T_sb[:, 0:DET - 1])
                        # TAB[:, k, 0] = T[k]
                        nc.scalar.copy(out=TAB_o[:, :, 0], in_=T_sb[:])
                        # TAB[:, k, 1] = E0[k] = Dp[k]
                        nc.scalar.copy(out=TAB_o[:, :, 1],
                                       in_=Dp_sb[:, 0:DET])
                        # TAB[:, k, 1+r] = Dp[k+r] - Dp[k+r-1], r=1..ntap-1
                        for r in range(1, ntap_o):
                            nc.vector.tensor_sub(TAB_o[:, :, 1 + r],
                                                 Dp_sb[:, r:DET + r],
                                                 Dp_sb[:, r - 1:DET + r - 1])

            # ---- gather + extract + accumulate
            with (
                tc.tile_pool(name="idx", bufs=8) as idxp,
                tc.tile_pool(name="pfp", bufs=3) as pfp,
                tc.tile_pool(name="sg", bufs=5) as sgp,
                tc.tile_pool(name="extv", bufs=3) as extp,
                tc.tile_pool(name="exts", bufs=2) as extsp,
                tc.tile_pool(name="gout", bufs=2) as gop,
                tc.tile_pool(name="ps_acc", bufs=2, space="PSUM") as psacc,
            ):
                def extract(sg, pf, tag, ntap, nseg, segw):
                    """val[c, j] = T[a] + sum_r relu(pf - r) * Er[a].

                    r=0: pf >= 0 so relu(pf) == pf (no relu needed).
                    r>0: relu(pf - r) == relu(relu(pf - r + 1) - 1), so the
                    relus chain off each other through two ping-pong planes.
                    All Er columns are expanded to per-pixel tiles in ONE
                    batched scalar copy so every DVE multiply runs on
                    contiguous fp16; the T-term folds in with a broadcast
                    add, halving the accumulation matmuls.
                    """
                    val = extp.tile([128, nseg, segw], f16, tag=f"val{tag}")
                    tmp = extsp.tile([128, nseg, segw], f16, tag=f"tmp{tag}")
                    Ra = extsp.tile([128, nseg, segw], f16, tag=f"Ra{tag}")
                    Rb = extsp.tile([128, nseg, segw], f16, tag=f"Rb{tag}")
                    EE = extsp.tile([128, nseg, ntap, segw], f16,
                                    tag=f"EE{tag}")
                    src = sg[:, :, 1:1 + ntap]
                    eb = AP(src.tensor, src.offset, [*src.ap, (0, segw)])
                    nc.scalar.copy(out=EE[:], in_=eb)
                    for r in range(ntap):
                        if r == 0:
                            R = pf
                        else:
                            # independent relus straight from pf (no chain:
                            # keeps the scalar queue off the vector critical
                            # path)
                            R = Ra if r % 2 == 1 else Rb
                            nc.scalar.activation(R[:], pf[:], Act.Relu,
                                                 bias=bias_t[:, r:r + 1])
                        dst = val if r == 0 else tmp
                        nc.vector.tensor_mul(dst[:], R[:], EE[:, :, r, :])
                        if r > 0:
                            nc.vector.tensor_add(val[:], val[:], tmp[:])
                    # fold the T-term in with a broadcast add (frees the PE
                    # from streaming it as a separate matmul operand)
                    _, tb = broadcast_tensor_aps(val[:], sg[:, :, 0:1])
                    nc.vector.tensor_add(val[:], val[:], tb)
                    return val

                def issue_dmas(cc):
                    """Input DMAs for iteration cc (pipelined one ahead)."""
                    it2 = idxp.tile([128, NSEG2 // 16], i16, tag="it2")
                    nc.sync.dma_start(out=it2[:], in_=idx2_d[cc])
                    pf2_t = pfp.tile([128, NSEG2, SEG2], f16, tag="pf2")
                    nc.sync.dma_start(out=pf2_t[:], in_=pf2_d[cc])
                    its = []
                    pfs = []
                    for h in range(2):
                        ci = 2 * cc + h
                        it = idxp.tile([128, NSEG // 16], i16, tag="it1")
                        nc.sync.dma_start(out=it[:], in_=idx1_d[ci])
                        pf1_t = pfp.tile([128, NSEG, SEG], f16, tag="pf1")
                        nc.sync.dma_start(out=pf1_t[:], in_=pf1_d[ci])
                        its.append(it)
                        pfs.append(pf1_t)
                    return it2, pf2_t, its, pfs

                # one PSUM tile whose four 32-partition slots rotate:
                # 4-deep accumulate/evict pipelining (each slot has its
                # own partitions' banks)
                bigacc = psacc.tile([128, CHUNK], f32)
                pending_dma = issue_dmas(0)
                pending = []
                for cc in range(NCC):
                    it2, pf2_t, its, pf1s = pending_dma
                    if cc + 1 < NCC:
                        pending_dma = issue_dmas(cc + 1)
                    # issue all three gathers up front so the gpsimd queue
                    # never stalls on the downstream extract pipeline
                    sg2 = sgp.tile([128, NSEG2, DW2], f16, tag="sg2")
                    nc.gpsimd.ap_gather(sg2[:], TAB2_t[:], it2[:],
                                        channels=128, num_elems=DET,
                                        d=DW2, num_idxs=NSEG2)
                    sg1s = []
                    for h in range(2):
                        sg1 = sgp.tile([128, NSEG, DW], f16, tag="sg1")
                        nc.gpsimd.ap_gather(sg1[:], TAB1_t[:], its[h][:],
                                            channels=128, num_elems=DET,
                                            d=DW, num_idxs=NSEG)
                        sg1s.append(sg1)
                    # evict the PREVIOUS iteration's PSUM accumulators now
                    # (their matmuls are long done) so the scalar queue never
                    # head-of-line blocks this iteration's expand/relu work
                    for acc_p, ci_p in pending:
                        ev = gop.tile([16, CHUNK], f16, tag="ev")
                        nc.scalar.copy(out=ev[:], in_=acc_p)
                        nc.sync.dma_start(out=out_d[ci_p], in_=ev[:])
                    pending = []
                    val2 = extract(sg2, pf2_t, "2", NTAP2, NSEG2, SEG2)
                    for h in range(2):
                        ci = 2 * cc + h
                        sg1 = sg1s[h]
                        val1 = extract(sg1, pf1s[h], "1", NTAP1, NSEG, SEG)
                        slot = (2 * cc + h) % 3
                        acc = bigacc[32 * slot:32 * slot + 16, :]
                        # per-region [start, stop] pairs adjacent (interleaved
                        # groups in a shared PSUM bank drop updates)
                        for j in range(4):
                            js = slice(128 * j, 128 * (j + 1))
                            j2 = slice(64 * j, 64 * (j + 1))
                            ps = acc[:, 512 * j:512 * (j + 1)]
                            nc.tensor.matmul(ps, selab_t[:, 0, :],
                                             val1[:, js, :],
                                             start=True, stop=False)
                            nc.tensor.matmul(ps, selab_t[:, 1 + h, :],
                                             val2[:, j2, :],
                                             start=False, stop=True)
                        pending.append((acc, ci))
                for acc_p, ci_p in pending:
                    ev = gop.tile([16, CHUNK], f16, tag="ev")
                    nc.scalar.copy(out=ev[:], in_=acc_p)
                    nc.sync.dma_start(out=out_d[ci_p], in_=ev[:])
    nc.compile()
    return nc


def _get_compiled():
    if "nc" not in _cache:
        (colmask, dftAc, dftAs, dftBc, dftBs, dftBcr, dftBsr, sela,
         ix_all) = _host_constants()
        _cache["consts"] = (colmask, dftAc, dftAs, dftBc, dftBs,
                            dftBcr, dftBsr, sela)
        _cache["percore"] = _per_core_inputs(ix_all)
        _cache["nc"] = _build_nc()
    return _cache["nc"], _cache["consts"], _cache["percore"]


def _in_maps(x, conv_w, ramlak, consts, percore):
    (colmask, dftAc, dftAs, dftBc, dftBs, dftBcr, dftBsr, sela) = consts
    x3 = np.ascontiguousarray(
        np.asarray(x, dtype=np.float32).reshape(NB, IMG, IMG))
    # w9[ki, kj, a] = conv_w[a, 0, ki, kj]; device tile partition axis = ki
    w9 = np.ascontiguousarray(
        np.asarray(conv_w, dtype=np.float32).reshape(NA, 3, 3).transpose(1, 2, 0))
    r = np.asarray(ramlak, dtype=np.float32)
    common = {
        "x3": x3, "w9": w9,
        "rcol": np.ascontiguousarray(r[0:128].reshape(128, 1)),
        "rnyq": np.ascontiguousarray(r[128:129].reshape(1, 1)),
        "colmask": colmask, "dftAc": dftAc, "dftAs": dftAs,
        "dftBc": dftBc, "dftBs": dftBs, "dftBcr": dftBcr, "dftBsr": dftBsr,
        "sela": np.ascontiguousarray(sela.transpose(1, 0, 2)),
    }
    in_maps = []
    for r_ in range(NCORES):
        s1n, s1r, s2n, s2r, idx1, idx2, pf1, pf2 = percore[r_]
        m = dict(common)
        m.update({"sel1n": s1n, "sel1r": s1r, "sel2n": s2n, "sel2r": s2r,
                  "idx1": idx1, "idx2": idx2, "pf1": pf1, "pf2": pf2})
        in_maps.append(m)
    return in_maps


def kernel(x, conv_w, ramlak):
    from concourse.bass_utils import run_bass_kernel_spmd

    nc, consts, percore = _get_compiled()
    in_maps = _in_maps(x, conv_w, ramlak, consts, percore)
    res = run_bass_kernel_spmd(nc, in_maps, list(range(NCORES)))

    totx = np.zeros((NCHP, 16, CHUNK), dtype=np.float32)
    toty = np.zeros((NCHP, 16, CHUNK), dtype=np.float32)
    for r_ in range(NCORES):
        part = res.results[r_]["partial"].astype(np.float32)
        if r_ < 4:
            totx += part
        else:
            toty += part
    img = np.zeros((2, NB, IMG, IMG), dtype=np.float32)
    # x-fast cores: rows m = 8*hi + 4*s + b; pixels u = 8 rows x 256 cols
    tx = totx.reshape(NCHP, 2, 2, NB, 8, IMG)
    for ci in range(NCHP):
        img[:, :, 8 * ci:8 * ci + 8, :] += tx[ci, 0]
        img[:, :, IMG - 8 * ci - 8:IMG - 8 * ci, :] += \
            tx[ci, 1][:, :, ::-1, ::-1]
    # y-fast cores: chunk ci = image columns 8ci..8ci+7, u = image row
    ty = toty.reshape(NCHP, 2, 2, NB, 8, IMG)
    for ci in range(NCHP):
        img[:, :, :, 8 * ci:8 * ci + 8] += ty[ci, 0].transpose(0, 1, 3, 2)
        img[:, :, :, IMG - 8 * ci - 8:IMG - 8 * ci] += \
            ty[ci, 1].transpose(0, 1, 3, 2)[:, :, ::-1, ::-1]
    direct = img[0]
    mirrored = img[1][:, :, ::-1]
    out = (direct + mirrored) * np.float32(np.pi / NA)
    return np.ascontiguousarray(out.reshape(NB, 1, IMG, IMG)).astype(np.float32)
